# revision 9
# baseline (speedup 1.0000x reference)
"""Causal multi-head attention (dense transformer block) on 8 Trainium2 cores.

Problem: x[4, 2048, 1024], 16 heads, head_dim 64, causal softmax attention
with QKV + output projections (torch Linear layout weights).

Sharding: 8 cores = 4 batches x 2 head-groups (8 heads each).  Each core
computes QKV projection for its 8 heads, attention, and its partial output
projection (row-parallel over w_out).  Host sums the two partials per batch
and adds b_out.

Device layouts are "transposed" so no on-device transposes are needed:
  - x is fed as xT [d, s]; Q^T/K^T are produced as [head_dim, s]
  - scores are computed as S^T [k, q]; the two heads of a pair run as
    row-group-tiled concurrent matmuls (K=64 contraction at array rows
    0-63 / 64-127).
  - AV is col-group packed: per head-pair one PSUM bank holds O^T for
    head A in partitions 0..63 and head B in partitions 64..127, written
    by two concurrent col-tiled matmuls (tile_position auto-derived).
  - softmax denominators come from four col-packed M=1 ones-matmuls per
    i-step accumulating into rows 0/32/64/96 of a dedicated PSUM bank.
  - normalization: denominator rows are copied (partition-remapped) to
    SBUF, reciprocal'd at [2,512] cost, broadcast into a full 128-row
    PSUM bank by one K=2 selector matmul per hp, copied to SBUF, and
    applied with one [128,512] DVE multiply per hp.
  - PE filler: the deferred Q-chunk projections (j>=1), deferred V tiles
    (st>=12), and the output projections are emitted *between* attention
    i-steps so the tensor engine never idles while the scalar engine
    (exp) catches up -- this also keeps the PE HAM clock un-throttled.
Matmul inputs are bf16 (PSUM accumulation is fp32); everything else fp32.
"""

import sys

sys.path.insert(0, "/opt/trn_rl_repo")

from collections import deque

import numpy as np
import ml_dtypes

import concourse.bass as bass
import concourse.mybir as mybir
import concourse.tile as tile
from concourse import bacc
from concourse import bass_utils
from concourse.masks import make_upper_triangular

F32 = mybir.dt.float32
BF16 = mybir.dt.bfloat16
EXP = mybir.ActivationFunctionType.Exp

B, S, D = 4, 2048, 1024
HTOT, HD = 16, 64
NCORES = 8
HLOC = HTOT // 2          # heads per core
ELOC = HLOC * HD          # 512 local embedding width
NHP = HLOC // 2           # 4 head pairs
QC = 512                  # q-chunk width
NQC = S // QC             # 4
NKT = S // 128            # 16 k tiles over sequence
NDT = D // 128            # 8 k tiles over model dim
SCALE = 1.0 / float(np.sqrt(HD))
NVUP = 12                 # V s-tiles computed upfront; the rest are filler

_CACHE = {}


def _build_nc():
    nc = bacc.Bacc("TRN2", target_bir_lowering=False, debug=False)

    xT = nc.dram_tensor("xT", [D, S], BF16, kind="ExternalInput")
    wqT = nc.dram_tensor("wqT", [D, ELOC], BF16, kind="ExternalInput")
    wkT = nc.dram_tensor("wkT", [D, ELOC], BF16, kind="ExternalInput")
    wvT = nc.dram_tensor("wvT", [D, ELOC], BF16, kind="ExternalInput")
    woT = nc.dram_tensor("woT", [ELOC, D], BF16, kind="ExternalInput")
    bqk = nc.dram_tensor("bqk", [128, 2, NHP], F32, kind="ExternalInput")
    bvb = nc.dram_tensor("bvb", [128, HLOC, HD], F32, kind="ExternalInput")
    outp = nc.dram_tensor("outp", [S, D], F32, kind="ExternalOutput")

    with tile.TileContext(nc) as tc:
        with tc.tile_pool(name="const", bufs=1) as constp, \
             tc.tile_pool(name="wpool", bufs=1) as wp, \
             tc.tile_pool(name="qkv", bufs=1) as qkvp, \
             tc.tile_pool(name="xt", bufs=1) as xtp, \
             tc.tile_pool(name="pt", bufs=8) as ptp, \
             tc.tile_pool(name="otn", bufs=8) as otnp, \
             tc.tile_pool(name="dr", bufs=8) as drp, \
             tc.tile_pool(name="rds", bufs=2) as rdsp, \
             tc.tile_pool(name="osb", bufs=4) as osbp:

            # ---- constants ----
            trimask = constp.tile([128, 128], BF16, name="trimask")
            make_upper_triangular(nc, trimask[:], val=1.0, diag=True)
            ones1 = constp.tile([128, 1], BF16, name="ones1")
            nc.gpsimd.memset(ones1[:], 1.0)
            # ones row for the K=1 denominator-broadcast matmuls
            onesr = constp.tile([1, 64], BF16, name="onesr")
            nc.gpsimd.memset(onesr[:], 1.0)

            # ---- weights + xT (V inputs first so compute starts early;
            # x is streamed in s-chunks interleaved with the wv tiles) ----
            wv_sb = [wp.tile([128, ELOC], BF16, name=f"wv{kt}")
                     for kt in range(NDT)]
            xts = [xtp.tile([128, S], BF16, name=f"xt{kt}")
                   for kt in range(NDT)]
            for kt in range(NDT):
                nc.sync.dma_start(wv_sb[kt][:],
                                  wvT[128 * kt:128 * (kt + 1), :])
                nc.sync.dma_start(
                    xts[kt][:, 0:512], xT[128 * kt:128 * (kt + 1), 0:512])
            bqk_sb = constp.tile([128, 2, NHP], F32, name="bqk_sb")
            nc.sync.dma_start(bqk_sb[:], bqk[:])
            bvb_sb = constp.tile([128, HLOC, HD], F32, name="bvb_sb")
            nc.sync.dma_start(bvb_sb[:], bvb[:])
            for c in range(1, 4):
                for kt in range(NDT):
                    nc.sync.dma_start(
                        xts[kt][:, 512 * c:512 * (c + 1)],
                        xT[128 * kt:128 * (kt + 1), 512 * c:512 * (c + 1)])
            wq_sb, wk_sb = [], []
            for kt in range(NDT):
                for lst, srct, nm in ((wq_sb, wqT, "wq"), (wk_sb, wkT, "wk")):
                    t = wp.tile([128, ELOC], BF16, name=f"{nm}{kt}")
                    nc.sync.dma_start(t[:], srct[128 * kt:128 * (kt + 1), :])
                    lst.append(t)
            wo_sb = []
            for hp in range(NHP):
                t = wp.tile([128, D], BF16, name=f"wo{hp}")
                nc.sync.dma_start(t[:], woT[128 * hp:128 * (hp + 1), :])
                wo_sb.append(t)

            # ---- QKV projection tiles ----
            QT, KT = [], []
            for hp in range(NHP):
                QT.append(qkvp.tile([128, S], BF16, name=f"qt{hp}"))
                KT.append(qkvp.tile([128, S], BF16, name=f"kt{hp}"))
            V = [qkvp.tile([128, HLOC, HD], BF16, name=f"v{st}")
                 for st in range(NKT)]

            def make_v(pool, st, tag=""):
                ps = pool.tile([128, HLOC, HD], F32, tag=tag or "psA", name="psv")
                for kt in range(NDT):
                    nc.tensor.matmul(
                        ps[:, :, :],
                        lhsT=xts[kt][:, 128 * st:128 * (st + 1)],
                        rhs=wv_sb[kt][:],
                        start=(kt == 0), stop=(kt == NDT - 1))
                nc.vector.tensor_add(V[st][:, :, :], ps[:, :, :],
                                     bvb_sb[:, :, :])

            def make_proj(pool, dst, wsb, col, hp, c, tag=""):
                # one 512-wide chunk of Q^T or K^T for head-pair hp
                ps = pool.tile([128, QC], F32, tag=tag or "psA", name="psp")
                for kt in range(NDT):
                    nc.tensor.matmul(
                        ps[:],
                        lhsT=wsb[kt][:, 128 * hp:128 * (hp + 1)],
                        rhs=xts[kt][:, QC * c:QC * (c + 1)],
                        start=(kt == 0), stop=(kt == NDT - 1))
                nc.vector.tensor_scalar_add(
                    dst[hp][:, QC * c:QC * (c + 1)], ps[:],
                    bqk_sb[:, col, hp:hp + 1])

            # ---- phase A: V (first NVUP tiles), all K^T, Q^T chunk 0 ----
            with tc.tile_pool(name="psq", bufs=6, space="PSUM") as psq:
                for st in range(NVUP):
                    make_v(psq, st)
                for hp in range(NHP):
                    for c in range(NQC):
                        make_proj(psq, KT, wk_sb, 1, hp, c)
                for hp in range(NHP):
                    make_proj(psq, QT, wq_sb, 0, hp, 0)

            # ---- phase B: attention with PE filler ----
            tri3 = trimask[:][:, None, :].broadcast_to([128, 2, 128])
            otn_store = {}
            filler = deque()

            with tc.tile_pool(name="pss", bufs=2, space="PSUM") as pss, \
                 tc.tile_pool(name="pst", bufs=2, space="PSUM") as pstp, \
                 tc.tile_pool(name="dnp", bufs=1, space="PSUM") as dnp, \
                 tc.tile_pool(name="flt", bufs=1, space="PSUM") as flt:

                def qt_unit(hp, c):
                    # two parts: kt 0-3 (allocates the PSUM tile) and
                    # kt 4-7 + bias add (closes the accumulation group)
                    box = {}

                    def part1():
                        ps = flt.tile([128, QC], F32, tag="flt", name="psp")
                        box["ps"] = ps
                        for kt in range(4):
                            nc.tensor.matmul(
                                ps[:],
                                lhsT=wq_sb[kt][:, 128 * hp:128 * (hp + 1)],
                                rhs=xts[kt][:, QC * c:QC * (c + 1)],
                                start=(kt == 0), stop=False)

                    def part2():
                        ps = box["ps"]
                        for kt in range(4, NDT):
                            nc.tensor.matmul(
                                ps[:],
                                lhsT=wq_sb[kt][:, 128 * hp:128 * (hp + 1)],
                                rhs=xts[kt][:, QC * c:QC * (c + 1)],
                                start=False, stop=(kt == NDT - 1))
                        nc.vector.tensor_scalar_add(
                            QT[hp][:, QC * c:QC * (c + 1)], ps[:],
                            bqk_sb[:, 0, hp:hp + 1])
                    return [(False, part1), (True, part2)]

                def v_unit(st):
                    box = {}

                    def part1():
                        ps = flt.tile([128, HLOC, HD], F32, tag="flt",
                                      name="psv")
                        box["ps"] = ps
                        for kt in range(4):
                            nc.tensor.matmul(
                                ps[:, :, :],
                                lhsT=xts[kt][:, 128 * st:128 * (st + 1)],
                                rhs=wv_sb[kt][:],
                                start=(kt == 0), stop=False)

                    def part2():
                        ps = box["ps"]
                        for kt in range(4, NDT):
                            nc.tensor.matmul(
                                ps[:, :, :],
                                lhsT=xts[kt][:, 128 * st:128 * (st + 1)],
                                rhs=wv_sb[kt][:],
                                start=False, stop=(kt == NDT - 1))
                        nc.vector.tensor_add(V[st][:, :, :], ps[:, :, :],
                                             bvb_sb[:, :, :])
                    return [(False, part1), (True, part2)]

                def oproj_unit(j, m, eo, pool=None):
                    box = {}
                    s0 = QC * j + 128 * m

                    def part1():
                        ps_o = (pool or flt).tile([128, 512], F32,
                                                  tag="flt" if pool is None
                                                  else "dnp", name="ps_o")
                        box["ps"] = ps_o
                        for hp in range(2):
                            nc.tensor.matmul(
                                ps_o[:],
                                lhsT=otn_store[(j, hp)][:,
                                                        128 * m:128 * (m + 1)],
                                rhs=wo_sb[hp][:, 512 * eo:512 * (eo + 1)],
                                start=(hp == 0), stop=False)

                    def part2():
                        ps_o = box["ps"]
                        for hp in range(2, NHP):
                            nc.tensor.matmul(
                                ps_o[:],
                                lhsT=otn_store[(j, hp)][:,
                                                        128 * m:128 * (m + 1)],
                                rhs=wo_sb[hp][:, 512 * eo:512 * (eo + 1)],
                                start=False, stop=(hp == NHP - 1))
                        osb = osbp.tile([128, 512], F32)
                        nc.vector.tensor_copy(osb[:], ps_o[:])
                        nc.sync.dma_start(
                            outp[s0:s0 + 128, 512 * eo:512 * (eo + 1)],
                            osb[:])
                    return [(False, part1), (True, part2)]

                def pump(n):
                    for _ in range(n):
                        if not filler:
                            return
                        filler.popleft()[1]()

                def pump_flush_open():
                    # finish any unit whose PSUM accumulation group is
                    # still open so normalization may allocate from flt
                    while filler and filler[0][0]:
                        filler.popleft()[1]()

                def chain(j, pair):
                    nkt = 4 * j + 4
                    hps = (2 * pair, 2 * pair + 1)
                    ps_t = {hp: pstp.tile([128, QC], F32, tag="pst",
                                          name=f"ps_t{hp}") for hp in hps}
                    denps = dnp.tile([128, QC], F32, tag="dnp", name="denps")
                    SKEW = 2
                    pts_hist = {}

                    def emit_av_hp(iv, hp):
                        wv_ = 128 * (iv - 4 * j) if iv >= 4 * j else 0
                        pt = pts_hist[iv][hp]
                        for h2 in range(2):
                            nc.tensor.matmul(
                                ps_t[hp][64 * h2:64 * (h2 + 1), wv_:QC],
                                lhsT=V[iv][:, 2 * hp + h2, :],
                                rhs=pt[:, h2, wv_:QC],
                                start=(iv == 0),
                                stop=(iv == nkt - 1))

                    def emit_den(iv):
                        wv_ = 128 * (iv - 4 * j) if iv >= 4 * j else 0
                        for qi, (hp, h2) in enumerate(
                                (hp, h2) for hp in hps for h2 in range(2)):
                            pt = pts_hist[iv][hp]
                            nc.tensor.matmul(
                                denps[32 * qi:32 * qi + 1, wv_:QC],
                                lhsT=ones1[:, 0:1],
                                rhs=pt[:, h2, wv_:QC],
                                start=(iv == 0),
                                stop=(iv == nkt - 1),
                                tile_position=(0, 32 * qi))
                        del pts_hist[iv]

                    for i in range(nkt):
                        w = 128 * (i - 4 * j) if i >= 4 * j else 0
                        pts = {}
                        # per-hp: scores group then the (skewed) AV group
                        # of the same hp, so PE work sits between the two
                        # score groups while ACT drains the previous step
                        for hp in hps:
                            ps_s = pss.tile([128, 2, QC], F32, tag="pss",
                                            name="ps_s")
                            for h2 in range(2):
                                nc.tensor.matmul(
                                    ps_s[:, h2, w:QC],
                                    lhsT=KT[hp][64 * h2:64 * (h2 + 1),
                                                128 * i:128 * (i + 1)],
                                    rhs=QT[hp][64 * h2:64 * (h2 + 1),
                                               QC * j + w:QC * (j + 1)],
                                    start=True, stop=True)
                            pt = ptp.tile([128, 2, QC], BF16, tag="pt",
                                          name="pt")
                            nc.scalar.activation(pt[:, :, w:QC],
                                                 ps_s[:, :, w:QC],
                                                 EXP, scale=SCALE)
                            if i >= 4 * j:
                                nc.vector.tensor_mul(
                                    pt[:, :, w:w + 128],
                                    pt[:, :, w:w + 128], tri3[:, :, :])
                            pts[hp] = pt
                            if i >= SKEW:
                                emit_av_hp(i - SKEW, hp)
                        pts_hist[i] = pts
                        if i >= SKEW:
                            emit_den(i - SKEW)
                        pump(1)
                    for iv in range(max(0, nkt - SKEW), nkt):
                        for hp in hps:
                            emit_av_hp(iv, hp)
                        emit_den(iv)
                    pump_flush_open()

                    # ---- normalization ----
                    # each denominator row -> its own [1,512] SBUF tile at
                    # partition 0 (32-aligned partition remap only), then
                    # reciprocal + bf16 cast, then one K=1 broadcast matmul
                    # per (hp, h2) into a full [128,512] PSUM bank.
                    quads = [(qi, hp, h2) for qi, (hp, h2) in enumerate(
                        (hp, h2) for hp in hps for h2 in range(2))]
                    den_rb = {}
                    for qi, hp, h2 in quads:
                        dsb = drp.tile([1, QC], F32, name="den_sb",
                                       tag="den")
                        nc.vector.tensor_copy(
                            dsb[:], denps[32 * qi:32 * qi + 1, :])
                        dr = drp.tile([1, QC], F32, name="den_r", tag="den")
                        nc.vector.reciprocal_approx_fast(dr[:], dsb[:])
                        drb = drp.tile([1, QC], BF16, name="den_rb",
                                       tag="den")
                        with nc.allow_low_precision(reason="denom"):
                            nc.vector.tensor_copy(drb[:], dr[:])
                        den_rb[hp, h2] = drb
                    for hp in hps:
                        otn = otnp.tile([128, QC], BF16, tag="otn",
                                        name="otn")
                        rdps = flt.tile([128, QC], F32, tag="flt",
                                        name="rdps")
                        for h2 in range(2):
                            nc.tensor.matmul(
                                rdps[64 * h2:64 * (h2 + 1), :],
                                lhsT=onesr[:],
                                rhs=den_rb[hp, h2][:],
                                start=True, stop=True)
                        rdsb = rdsp.tile([128, QC], F32, name="rdsb")
                        nc.vector.tensor_copy(rdsb[:], rdps[:])
                        nc.vector.tensor_mul(otn[:], ps_t[hp][:], rdsb[:])
                        otn_store[(j, hp)] = otn

                for j in range(NQC):
                    if j + 1 < NQC:
                        for hp in range(NHP):
                            filler.extend(qt_unit(hp, j + 1))
                    if j == 3:
                        for st in range(NVUP, NKT):
                            filler.extend(v_unit(st))
                    chain(j, 0)
                    if j >= 1:
                        for m in range(4):
                            for eo in range(2):
                                filler.extend(oproj_unit(j - 1, m, eo))
                    chain(j, 1)
                # drain remaining filler + final output projection
                # (alternate the two free PSUM pools to halve the tail
                # serialization on the single flt bank)
                while filler:
                    filler.popleft()[1]()
                for k, (m, eo) in enumerate(
                        (m, eo) for m in range(4) for eo in range(2)):
                    for _, part in oproj_unit(3, m, eo,
                                              pool=dnp if k % 2 else None):
                        part()
    nc.compile()
    return nc


def _get_nc():
    if "nc" not in _CACHE:
        _CACHE["nc"] = _build_nc()
    return _CACHE["nc"]


def _prep_core_inputs(x, w_qkv, b_qkv, w_out, b, hg):
    r0 = ELOC * hg
    wq = w_qkv[r0:r0 + ELOC, :]
    wk = w_qkv[D + r0:D + r0 + ELOC, :]
    wv = w_qkv[2 * D + r0:2 * D + r0 + ELOC, :]
    bq = b_qkv[r0:r0 + ELOC]
    bk = b_qkv[D + r0:D + r0 + ELOC]
    bv = b_qkv[2 * D + r0:2 * D + r0 + ELOC]

    bf = ml_dtypes.bfloat16
    bqk_arr = np.empty((128, 2, NHP), np.float32)
    bqk_arr[:, 0, :] = bq.reshape(NHP, 128).T
    bqk_arr[:, 1, :] = bk.reshape(NHP, 128).T
    return {
        "xT": np.ascontiguousarray(x[b].T).astype(bf),
        "wqT": np.ascontiguousarray(wq.T).astype(bf),
        "wkT": np.ascontiguousarray(wk.T).astype(bf),
        "wvT": np.ascontiguousarray(wv.T).astype(bf),
        "woT": np.ascontiguousarray(w_out[:, r0:r0 + ELOC].T).astype(bf),
        "bqk": bqk_arr,
        "bvb": np.tile(bv.astype(np.float32)[None, :],
                       (128, 1)).reshape(128, HLOC, HD),
    }


def kernel(x, w_qkv, b_qkv, w_out, b_out, _trace=False, _trace_kwargs=None):
    x = np.asarray(x, np.float32)
    w_qkv = np.asarray(w_qkv, np.float32)
    b_qkv = np.asarray(b_qkv, np.float32)
    w_out = np.asarray(w_out, np.float32)
    b_out = np.asarray(b_out, np.float32)

    nc = _get_nc()
    in_maps = []
    for core in range(NCORES):
        b, hg = core // 2, core % 2
        in_maps.append(_prep_core_inputs(x, w_qkv, b_qkv, w_out, b, hg))

    kw = {}
    if _trace:
        kw.update(trace=True, **(_trace_kwargs or {}))
    import time
    res = None
    for attempt in range(4):
        try:
            res = bass_utils.run_bass_kernel_spmd(
                nc, in_maps, core_ids=list(range(NCORES)), **kw)
            break
        except Exception:
            if attempt == 3:
                raise
            # Transient axon/NRT device flake: reset the PJRT backend so the
            # retry starts from a clean client, like a fresh process would.
            try:
                import jax
                jax.clear_caches()
                import jax._src.xla_bridge as _xb
                _xb._clear_backends()
            except Exception:
                pass
            time.sleep(5.0 * (attempt + 1))

    out = np.empty((B, S, D), np.float32)
    for b in range(B):
        out[b] = res.results[2 * b]["outp"] + res.results[2 * b + 1]["outp"] \
            + b_out[None, :]
    if _trace:
        return out, res
    return out


# revision 11
# speedup vs baseline: 1.0085x; 1.0085x over previous
"""Causal multi-head attention (dense transformer block) on 8 Trainium2 cores.

Problem: x[4, 2048, 1024], 16 heads, head_dim 64, causal softmax attention
with QKV + output projections (torch Linear layout weights).

Sharding: 8 cores = 4 batches x 2 head-groups (8 heads each).  Each core
computes QKV projection for its 8 heads, attention, and its partial output
projection (row-parallel over w_out).  Host sums the two partials per batch
and adds b_out.

Device layouts are "transposed" so no on-device transposes are needed:
  - x is fed as xT [d, s]; Q^T/K^T are produced as [head_dim, s]
  - scores are computed as S^T [k, q]; the two heads of a pair run as
    row-group-tiled concurrent matmuls (K=64 contraction at array rows
    0-63 / 64-127).
  - AV is col-group packed: per head-pair one PSUM bank holds O^T for
    head A in partitions 0..63 and head B in partitions 64..127, written
    by two concurrent col-tiled matmuls (tile_position auto-derived).
  - softmax denominators come from four col-packed M=1 ones-matmuls per
    i-step accumulating into rows 0/32/64/96 of a dedicated PSUM bank.
  - normalization: denominator rows are copied (partition-remapped) to
    SBUF, reciprocal'd at [2,512] cost, broadcast into a full 128-row
    PSUM bank by one K=2 selector matmul per hp, copied to SBUF, and
    applied with one [128,512] DVE multiply per hp.
  - PE filler: the deferred Q-chunk projections (j>=1), deferred V tiles
    (st>=12), and the output projections are emitted *between* attention
    i-steps so the tensor engine never idles while the scalar engine
    (exp) catches up -- this also keeps the PE HAM clock un-throttled.
Matmul inputs are bf16 (PSUM accumulation is fp32); everything else fp32.
"""

import sys

sys.path.insert(0, "/opt/trn_rl_repo")

from collections import deque

import numpy as np
import ml_dtypes

import concourse.bass as bass
import concourse.mybir as mybir
import concourse.tile as tile
from concourse import bacc
from concourse import bass_utils
from concourse.masks import make_upper_triangular

F32 = mybir.dt.float32
BF16 = mybir.dt.bfloat16
EXP = mybir.ActivationFunctionType.Exp

B, S, D = 4, 2048, 1024
HTOT, HD = 16, 64
NCORES = 8
HLOC = HTOT // 2          # heads per core
ELOC = HLOC * HD          # 512 local embedding width
NHP = HLOC // 2           # 4 head pairs
QC = 512                  # q-chunk width
NQC = S // QC             # 4
NKT = S // 128            # 16 k tiles over sequence
NDT = D // 128            # 8 k tiles over model dim
SCALE = 1.0 / float(np.sqrt(HD))
NVUP = 12                 # V s-tiles computed upfront; the rest are filler

_CACHE = {}


def _build_nc():
    nc = bacc.Bacc("TRN2", target_bir_lowering=False, debug=False)

    xT = nc.dram_tensor("xT", [D, S], BF16, kind="ExternalInput")
    wqT = nc.dram_tensor("wqT", [D, ELOC], BF16, kind="ExternalInput")
    wkT = nc.dram_tensor("wkT", [D, ELOC], BF16, kind="ExternalInput")
    wvT = nc.dram_tensor("wvT", [D, ELOC], BF16, kind="ExternalInput")
    woT = nc.dram_tensor("woT", [ELOC, D], BF16, kind="ExternalInput")
    bqk = nc.dram_tensor("bqk", [128, 2, NHP], F32, kind="ExternalInput")
    bvb = nc.dram_tensor("bvb", [128, HLOC, HD], F32, kind="ExternalInput")
    outp = nc.dram_tensor("outp", [S, D], F32, kind="ExternalOutput")

    with tile.TileContext(nc) as tc:
        with tc.tile_pool(name="const", bufs=1) as constp, \
             tc.tile_pool(name="wpool", bufs=1) as wp, \
             tc.tile_pool(name="qkv", bufs=1) as qkvp, \
             tc.tile_pool(name="xt", bufs=1) as xtp, \
             tc.tile_pool(name="pt", bufs=8) as ptp, \
             tc.tile_pool(name="otn", bufs=8) as otnp, \
             tc.tile_pool(name="dr", bufs=8) as drp, \
             tc.tile_pool(name="rds", bufs=2) as rdsp, \
             tc.tile_pool(name="osb", bufs=4) as osbp:

            # ---- constants ----
            trimask = constp.tile([128, 128], BF16, name="trimask")
            make_upper_triangular(nc, trimask[:], val=1.0, diag=True)
            ones1 = constp.tile([128, 1], BF16, name="ones1")
            nc.gpsimd.memset(ones1[:], 1.0)
            # ones row for the K=1 denominator-broadcast matmuls
            onesr = constp.tile([1, 64], BF16, name="onesr")
            nc.gpsimd.memset(onesr[:], 1.0)

            # ---- weights + xT (V inputs first so compute starts early;
            # x is streamed in s-chunks interleaved with the wv tiles) ----
            wv_sb = [wp.tile([128, ELOC], BF16, name=f"wv{kt}")
                     for kt in range(NDT)]
            xts = [xtp.tile([128, S], BF16, name=f"xt{kt}")
                   for kt in range(NDT)]
            for kt in range(NDT):
                nc.sync.dma_start(wv_sb[kt][:],
                                  wvT[128 * kt:128 * (kt + 1), :])
                nc.sync.dma_start(
                    xts[kt][:, 0:512], xT[128 * kt:128 * (kt + 1), 0:512])
            bqk_sb = constp.tile([128, 2, NHP], F32, name="bqk_sb")
            nc.sync.dma_start(bqk_sb[:], bqk[:])
            bvb_sb = constp.tile([128, HLOC, HD], F32, name="bvb_sb")
            nc.sync.dma_start(bvb_sb[:], bvb[:])
            for c in range(1, 4):
                for kt in range(NDT):
                    nc.sync.dma_start(
                        xts[kt][:, 512 * c:512 * (c + 1)],
                        xT[128 * kt:128 * (kt + 1), 512 * c:512 * (c + 1)])
            wq_sb, wk_sb = [], []
            for kt in range(NDT):
                for lst, srct, nm in ((wq_sb, wqT, "wq"), (wk_sb, wkT, "wk")):
                    t = wp.tile([128, ELOC], BF16, name=f"{nm}{kt}")
                    nc.sync.dma_start(t[:], srct[128 * kt:128 * (kt + 1), :])
                    lst.append(t)
            wo_sb = []
            for hp in range(NHP):
                t = wp.tile([128, D], BF16, name=f"wo{hp}")
                nc.sync.dma_start(t[:], woT[128 * hp:128 * (hp + 1), :])
                wo_sb.append(t)

            # ---- QKV projection tiles ----
            QT, KT = [], []
            for hp in range(NHP):
                QT.append(qkvp.tile([128, S], BF16, name=f"qt{hp}"))
                KT.append(qkvp.tile([128, S], BF16, name=f"kt{hp}"))
            V = [qkvp.tile([128, HLOC, HD], BF16, name=f"v{st}")
                 for st in range(NKT)]

            def make_v(pool, st, tag=""):
                ps = pool.tile([128, HLOC, HD], F32, tag=tag or "psA", name="psv")
                for kt in range(NDT):
                    nc.tensor.matmul(
                        ps[:, :, :],
                        lhsT=xts[kt][:, 128 * st:128 * (st + 1)],
                        rhs=wv_sb[kt][:],
                        start=(kt == 0), stop=(kt == NDT - 1))
                nc.vector.tensor_add(V[st][:, :, :], ps[:, :, :],
                                     bvb_sb[:, :, :])

            def make_proj(pool, dst, wsb, col, hp, c, tag=""):
                # one 512-wide chunk of Q^T or K^T for head-pair hp
                ps = pool.tile([128, QC], F32, tag=tag or "psA", name="psp")
                for kt in range(NDT):
                    nc.tensor.matmul(
                        ps[:],
                        lhsT=wsb[kt][:, 128 * hp:128 * (hp + 1)],
                        rhs=xts[kt][:, QC * c:QC * (c + 1)],
                        start=(kt == 0), stop=(kt == NDT - 1))
                nc.vector.tensor_scalar_add(
                    dst[hp][:, QC * c:QC * (c + 1)], ps[:],
                    bqk_sb[:, col, hp:hp + 1])

            # ---- phase A: V (first NVUP tiles), all K^T, Q^T chunk 0 ----
            with tc.tile_pool(name="psq", bufs=6, space="PSUM") as psq:
                for st in range(NVUP):
                    make_v(psq, st)
                for hp in range(NHP):
                    for c in range(NQC):
                        make_proj(psq, KT, wk_sb, 1, hp, c)
                for hp in range(NHP):
                    make_proj(psq, QT, wq_sb, 0, hp, 0)

            # ---- phase B: attention with PE filler ----
            tri3 = trimask[:][:, None, :].broadcast_to([128, 2, 128])
            otn_store = {}
            filler = deque()

            with tc.tile_pool(name="pss", bufs=2, space="PSUM") as pss, \
                 tc.tile_pool(name="pst", bufs=2, space="PSUM") as pstp, \
                 tc.tile_pool(name="dnp", bufs=1, space="PSUM") as dnp, \
                 tc.tile_pool(name="flt", bufs=1, space="PSUM") as flt:

                def qt_unit(hp, c):
                    # two parts: kt 0-3 (allocates the PSUM tile) and
                    # kt 4-7 + bias add (closes the accumulation group)
                    box = {}

                    def part1():
                        ps = flt.tile([128, QC], F32, tag="flt", name="psp")
                        box["ps"] = ps
                        for kt in range(4):
                            nc.tensor.matmul(
                                ps[:],
                                lhsT=wq_sb[kt][:, 128 * hp:128 * (hp + 1)],
                                rhs=xts[kt][:, QC * c:QC * (c + 1)],
                                start=(kt == 0), stop=False)

                    def part2():
                        ps = box["ps"]
                        for kt in range(4, NDT):
                            nc.tensor.matmul(
                                ps[:],
                                lhsT=wq_sb[kt][:, 128 * hp:128 * (hp + 1)],
                                rhs=xts[kt][:, QC * c:QC * (c + 1)],
                                start=False, stop=(kt == NDT - 1))
                        nc.vector.tensor_scalar_add(
                            QT[hp][:, QC * c:QC * (c + 1)], ps[:],
                            bqk_sb[:, 0, hp:hp + 1])
                    return [(False, part1), (True, part2)]

                def v_unit(st):
                    box = {}

                    def part1():
                        ps = flt.tile([128, HLOC, HD], F32, tag="flt",
                                      name="psv")
                        box["ps"] = ps
                        for kt in range(4):
                            nc.tensor.matmul(
                                ps[:, :, :],
                                lhsT=xts[kt][:, 128 * st:128 * (st + 1)],
                                rhs=wv_sb[kt][:],
                                start=(kt == 0), stop=False)

                    def part2():
                        ps = box["ps"]
                        for kt in range(4, NDT):
                            nc.tensor.matmul(
                                ps[:, :, :],
                                lhsT=xts[kt][:, 128 * st:128 * (st + 1)],
                                rhs=wv_sb[kt][:],
                                start=False, stop=(kt == NDT - 1))
                        nc.vector.tensor_add(V[st][:, :, :], ps[:, :, :],
                                             bvb_sb[:, :, :])
                    return [(False, part1), (True, part2)]

                def oproj_unit(j, m, eo, pool=None):
                    s0 = QC * j + 128 * m

                    def go():
                        ps_o = (pool or flt).tile([128, 512], F32,
                                                  tag="flt" if pool is None
                                                  else "dnp", name="ps_o")
                        for hp in range(NHP):
                            nc.tensor.matmul(
                                ps_o[:],
                                lhsT=otn_store[(j, hp)][:,
                                                        128 * m:128 * (m + 1)],
                                rhs=wo_sb[hp][:, 512 * eo:512 * (eo + 1)],
                                start=(hp == 0), stop=(hp == NHP - 1))
                        osb = osbp.tile([128, 512], F32)
                        nc.vector.tensor_copy(osb[:], ps_o[:])
                        nc.sync.dma_start(
                            outp[s0:s0 + 128, 512 * eo:512 * (eo + 1)],
                            osb[:])
                    return [(False, go)]

                def pump(n):
                    for _ in range(n):
                        if not filler:
                            return
                        filler.popleft()[1]()

                def pump_flush_open():
                    # finish any unit whose PSUM accumulation group is
                    # still open so normalization may allocate from flt
                    while filler and filler[0][0]:
                        filler.popleft()[1]()

                def pump_unit():
                    # pop one FULL unit (never leaves a flt group open;
                    # safe to call between flt allocations)
                    if filler:
                        filler.popleft()[1]()
                    while filler and filler[0][0]:
                        filler.popleft()[1]()

                def chain(j, pair):
                    nkt = 4 * j + 4
                    hps = (2 * pair, 2 * pair + 1)
                    ps_t = {hp: pstp.tile([128, QC], F32, tag="pst",
                                          name=f"ps_t{hp}") for hp in hps}
                    denps = dnp.tile([128, QC], F32, tag="dnp", name="denps")
                    SKEW = 2
                    pts_hist = {}

                    def emit_av_hp(iv, hp):
                        wv_ = 128 * (iv - 4 * j) if iv >= 4 * j else 0
                        pt = pts_hist[iv][hp]
                        for h2 in range(2):
                            nc.tensor.matmul(
                                ps_t[hp][64 * h2:64 * (h2 + 1), wv_:QC],
                                lhsT=V[iv][:, 2 * hp + h2, :],
                                rhs=pt[:, h2, wv_:QC],
                                start=(iv == 0),
                                stop=(iv == nkt - 1))

                    def emit_den(iv):
                        wv_ = 128 * (iv - 4 * j) if iv >= 4 * j else 0
                        for qi, (hp, h2) in enumerate(
                                (hp, h2) for hp in hps for h2 in range(2)):
                            pt = pts_hist[iv][hp]
                            nc.tensor.matmul(
                                denps[32 * qi:32 * qi + 1, wv_:QC],
                                lhsT=ones1[:, 0:1],
                                rhs=pt[:, h2, wv_:QC],
                                start=(iv == 0),
                                stop=(iv == nkt - 1),
                                tile_position=(0, 32 * qi))
                        del pts_hist[iv]

                    for i in range(nkt):
                        w = 128 * (i - 4 * j) if i >= 4 * j else 0
                        pts = {}
                        for hp in hps:
                            ps_s = pss.tile([128, 2, QC], F32, tag="pss",
                                            name="ps_s")
                            for h2 in range(2):
                                nc.tensor.matmul(
                                    ps_s[:, h2, w:QC],
                                    lhsT=KT[hp][64 * h2:64 * (h2 + 1),
                                                128 * i:128 * (i + 1)],
                                    rhs=QT[hp][64 * h2:64 * (h2 + 1),
                                               QC * j + w:QC * (j + 1)],
                                    start=True, stop=True)
                            pt = ptp.tile([128, 2, QC], BF16, tag="pt",
                                          name="pt")
                            nc.scalar.activation(pt[:, :, w:QC],
                                                 ps_s[:, :, w:QC],
                                                 EXP, scale=SCALE)
                            if i >= 4 * j:
                                nc.vector.tensor_mul(
                                    pt[:, :, w:w + 128],
                                    pt[:, :, w:w + 128], tri3[:, :, :])
                            pts[hp] = pt
                        pts_hist[i] = pts
                        if i >= SKEW:
                            for hp in hps:
                                emit_av_hp(i - SKEW, hp)
                            emit_den(i - SKEW)
                        pump(1)
                    for iv in range(max(0, nkt - SKEW), nkt):
                        for hp in hps:
                            emit_av_hp(iv, hp)
                        emit_den(iv)
                    pump_flush_open()

                    # ---- normalization ----
                    # each denominator row -> its own [1,512] SBUF tile at
                    # partition 0 (32-aligned partition remap only), then
                    # reciprocal + bf16 cast, then one K=1 broadcast matmul
                    # per (hp, h2) into a full [128,512] PSUM bank.
                    quads = [(qi, hp, h2) for qi, (hp, h2) in enumerate(
                        (hp, h2) for hp in hps for h2 in range(2))]
                    den_rb = {}
                    for qi, hp, h2 in quads:
                        dsb = drp.tile([1, QC], F32, name="den_sb",
                                       tag="den")
                        nc.vector.tensor_copy(
                            dsb[:], denps[32 * qi:32 * qi + 1, :])
                        dr = drp.tile([1, QC], F32, name="den_r", tag="den")
                        nc.vector.reciprocal_approx_fast(dr[:], dsb[:])
                        drb = drp.tile([1, QC], BF16, name="den_rb",
                                       tag="den")
                        with nc.allow_low_precision(reason="denom"):
                            nc.vector.tensor_copy(drb[:], dr[:])
                        den_rb[hp, h2] = drb
                    pump_unit()
                    for hp in hps:
                        otn = otnp.tile([128, QC], BF16, tag="otn",
                                        name="otn")
                        rdps = flt.tile([128, QC], F32, tag="flt",
                                        name="rdps")
                        for h2 in range(2):
                            nc.tensor.matmul(
                                rdps[64 * h2:64 * (h2 + 1), :],
                                lhsT=onesr[:],
                                rhs=den_rb[hp, h2][:],
                                start=True, stop=True)
                        rdsb = rdsp.tile([128, QC], F32, name="rdsb")
                        nc.vector.tensor_copy(rdsb[:], rdps[:])
                        nc.vector.tensor_mul(otn[:], ps_t[hp][:], rdsb[:])
                        otn_store[(j, hp)] = otn
                        pump_unit()

                for j in range(NQC):
                    if j + 1 < NQC:
                        for hp in range(NHP):
                            filler.extend(qt_unit(hp, j + 1))
                    if j == 3:
                        for st in range(NVUP, NKT):
                            filler.extend(v_unit(st))
                    chain(j, 0)
                    if j >= 1:
                        for m in range(4):
                            for eo in range(2):
                                filler.extend(oproj_unit(j - 1, m, eo))
                    chain(j, 1)
                # drain remaining filler + final output projection
                # (alternate the two free PSUM pools to halve the tail
                # serialization on the single flt bank)
                while filler:
                    filler.popleft()[1]()
                for k, (m, eo) in enumerate(
                        (m, eo) for m in range(4) for eo in range(2)):
                    for _, part in oproj_unit(3, m, eo,
                                              pool=dnp if k % 2 else None):
                        part()
    nc.compile()
    return nc


def _get_nc():
    if "nc" not in _CACHE:
        _CACHE["nc"] = _build_nc()
    return _CACHE["nc"]


def _prep_core_inputs(x, w_qkv, b_qkv, w_out, b, hg):
    r0 = ELOC * hg
    wq = w_qkv[r0:r0 + ELOC, :]
    wk = w_qkv[D + r0:D + r0 + ELOC, :]
    wv = w_qkv[2 * D + r0:2 * D + r0 + ELOC, :]
    bq = b_qkv[r0:r0 + ELOC]
    bk = b_qkv[D + r0:D + r0 + ELOC]
    bv = b_qkv[2 * D + r0:2 * D + r0 + ELOC]

    bf = ml_dtypes.bfloat16
    bqk_arr = np.empty((128, 2, NHP), np.float32)
    bqk_arr[:, 0, :] = bq.reshape(NHP, 128).T
    bqk_arr[:, 1, :] = bk.reshape(NHP, 128).T
    return {
        "xT": np.ascontiguousarray(x[b].T).astype(bf),
        "wqT": np.ascontiguousarray(wq.T).astype(bf),
        "wkT": np.ascontiguousarray(wk.T).astype(bf),
        "wvT": np.ascontiguousarray(wv.T).astype(bf),
        "woT": np.ascontiguousarray(w_out[:, r0:r0 + ELOC].T).astype(bf),
        "bqk": bqk_arr,
        "bvb": np.tile(bv.astype(np.float32)[None, :],
                       (128, 1)).reshape(128, HLOC, HD),
    }


def kernel(x, w_qkv, b_qkv, w_out, b_out, _trace=False, _trace_kwargs=None):
    x = np.asarray(x, np.float32)
    w_qkv = np.asarray(w_qkv, np.float32)
    b_qkv = np.asarray(b_qkv, np.float32)
    w_out = np.asarray(w_out, np.float32)
    b_out = np.asarray(b_out, np.float32)

    nc = _get_nc()
    in_maps = []
    for core in range(NCORES):
        b, hg = core // 2, core % 2
        in_maps.append(_prep_core_inputs(x, w_qkv, b_qkv, w_out, b, hg))

    kw = {}
    if _trace:
        kw.update(trace=True, **(_trace_kwargs or {}))
    import time
    res = None
    for attempt in range(4):
        try:
            res = bass_utils.run_bass_kernel_spmd(
                nc, in_maps, core_ids=list(range(NCORES)), **kw)
            break
        except Exception:
            if attempt == 3:
                raise
            # Transient axon/NRT device flake: reset the PJRT backend so the
            # retry starts from a clean client, like a fresh process would.
            try:
                import jax
                jax.clear_caches()
                import jax._src.xla_bridge as _xb
                _xb._clear_backends()
            except Exception:
                pass
            time.sleep(5.0 * (attempt + 1))

    out = np.empty((B, S, D), np.float32)
    for b in range(B):
        out[b] = res.results[2 * b]["outp"] + res.results[2 * b + 1]["outp"] \
            + b_out[None, :]
    if _trace:
        return out, res
    return out


# revision 13
# speedup vs baseline: 1.2559x; 1.2454x over previous
"""Causal multi-head attention (dense transformer block) on 8 Trainium2 cores.

Problem: x[4, 2048, 1024], 16 heads, head_dim 64, causal softmax attention
with QKV + output projections (torch Linear layout weights).

Sharding: 8 cores = 4 batches x 2 head-groups (8 heads each).  Each core
computes QKV projection for its 8 heads, attention, and its partial output
projection (row-parallel over w_out).  Host sums the two partials per batch
and adds b_out.

Device layouts are "transposed" so no on-device transposes are needed:
  - x is fed as xT [d, s]; Q^T/K^T are produced as [head_dim, s]
  - scores are computed as S^T [k, q]; the two heads of a pair run as
    row-group-tiled concurrent matmuls (K=64 contraction at array rows
    0-63 / 64-127).
  - AV is col-group packed: per head-pair one PSUM bank holds O^T for
    head A in partitions 0..63 and head B in partitions 64..127, written
    by two concurrent col-tiled matmuls (tile_position auto-derived).
  - softmax denominators come from four col-packed M=1 ones-matmuls per
    i-step accumulating into rows 0/32/64/96 of a dedicated PSUM bank.
  - normalization: denominator rows are copied (partition-remapped) to
    SBUF, reciprocal'd at [2,512] cost, broadcast into a full 128-row
    PSUM bank by one K=2 selector matmul per hp, copied to SBUF, and
    applied with one [128,512] DVE multiply per hp.
  - PE filler: the deferred Q-chunk projections (j>=1), deferred V tiles
    (st>=12), and the output projections are emitted *between* attention
    i-steps so the tensor engine never idles while the scalar engine
    (exp) catches up -- this also keeps the PE HAM clock un-throttled.
Matmul inputs are bf16 (PSUM accumulation is fp32); everything else fp32.
"""

import sys

sys.path.insert(0, "/opt/trn_rl_repo")

from collections import deque

import numpy as np
import ml_dtypes

import concourse.bass as bass
import concourse.mybir as mybir
import concourse.tile as tile
from concourse import bacc
from concourse import bass_utils
from concourse.masks import make_upper_triangular

F32 = mybir.dt.float32
BF16 = mybir.dt.bfloat16
EXP = mybir.ActivationFunctionType.Exp

B, S, D = 4, 2048, 1024
HTOT, HD = 16, 64
NCORES = 8
HLOC = HTOT // 2          # heads per core
ELOC = HLOC * HD          # 512 local embedding width
NHP = HLOC // 2           # 4 head pairs
QC = 512                  # q-chunk width
NQC = S // QC             # 4
NKT = S // 128            # 16 k tiles over sequence
NDT = D // 128            # 8 k tiles over model dim
SCALE = 1.0 / float(np.sqrt(HD))
NVUP = 12                 # V s-tiles computed upfront; the rest are filler

_CACHE = {}


def _build_nc():
    nc = bacc.Bacc("TRN2", target_bir_lowering=False, debug=False)

    xT = nc.dram_tensor("xT", [D, S], BF16, kind="ExternalInput")
    wqT = nc.dram_tensor("wqT", [D, ELOC], BF16, kind="ExternalInput")
    wkT = nc.dram_tensor("wkT", [D, ELOC], BF16, kind="ExternalInput")
    wvT = nc.dram_tensor("wvT", [D, ELOC], BF16, kind="ExternalInput")
    woT = nc.dram_tensor("woT", [ELOC, D], BF16, kind="ExternalInput")
    bqk = nc.dram_tensor("bqk", [128, 2, NHP], F32, kind="ExternalInput")
    bvb = nc.dram_tensor("bvb", [128, HLOC, HD], F32, kind="ExternalInput")
    outp = nc.dram_tensor("outp", [S, D], F32, kind="ExternalOutput")

    with tile.TileContext(nc) as tc:
        with tc.tile_pool(name="const", bufs=1) as constp, \
             tc.tile_pool(name="wpool", bufs=1) as wp, \
             tc.tile_pool(name="qkv", bufs=1) as qkvp, \
             tc.tile_pool(name="xt", bufs=1) as xtp, \
             tc.tile_pool(name="pt", bufs=8) as ptp, \
             tc.tile_pool(name="otn", bufs=8) as otnp, \
             tc.tile_pool(name="dr", bufs=8) as drp, \
             tc.tile_pool(name="rds", bufs=2) as rdsp, \
             tc.tile_pool(name="osb", bufs=4) as osbp:

            # ---- constants ----
            trimask = constp.tile([128, 128], BF16, name="trimask")
            make_upper_triangular(nc, trimask[:], val=1.0, diag=True)
            ones1 = constp.tile([128, 1], BF16, name="ones1")
            nc.gpsimd.memset(ones1[:], 1.0)
            # ones row for the K=1 denominator-broadcast matmuls
            onesr = constp.tile([1, 64], BF16, name="onesr")
            nc.gpsimd.memset(onesr[:], 1.0)

            # ---- weights + xT (V inputs first so compute starts early;
            # x is streamed in s-chunks interleaved with the wv tiles) ----
            wv_sb = [wp.tile([128, ELOC], BF16, name=f"wv{kt}")
                     for kt in range(NDT)]
            xts = [xtp.tile([128, S], BF16, name=f"xt{kt}")
                   for kt in range(NDT)]
            for kt in range(NDT):
                nc.sync.dma_start(wv_sb[kt][:],
                                  wvT[128 * kt:128 * (kt + 1), :])
                nc.sync.dma_start(
                    xts[kt][:, 0:512], xT[128 * kt:128 * (kt + 1), 0:512])
            bqk_sb = constp.tile([128, 2, NHP], F32, name="bqk_sb")
            nc.sync.dma_start(bqk_sb[:], bqk[:])
            bvb_sb = constp.tile([128, HLOC, HD], F32, name="bvb_sb")
            nc.sync.dma_start(bvb_sb[:], bvb[:])
            for c in range(1, 4):
                for kt in range(NDT):
                    nc.sync.dma_start(
                        xts[kt][:, 512 * c:512 * (c + 1)],
                        xT[128 * kt:128 * (kt + 1), 512 * c:512 * (c + 1)])
            wq_sb, wk_sb = [], []
            for kt in range(NDT):
                for lst, srct, nm in ((wq_sb, wqT, "wq"), (wk_sb, wkT, "wk")):
                    t = wp.tile([128, ELOC], BF16, name=f"{nm}{kt}")
                    nc.sync.dma_start(t[:], srct[128 * kt:128 * (kt + 1), :])
                    lst.append(t)
            wo_sb = []
            for hp in range(NHP):
                t = wp.tile([128, D], BF16, name=f"wo{hp}")
                nc.sync.dma_start(t[:], woT[128 * hp:128 * (hp + 1), :])
                wo_sb.append(t)

            # ---- QKV projection tiles ----
            QT, KT = [], []
            for hp in range(NHP):
                QT.append(qkvp.tile([128, S], BF16, name=f"qt{hp}"))
                KT.append(qkvp.tile([128, S], BF16, name=f"kt{hp}"))
            V = [qkvp.tile([128, HLOC, HD], BF16, name=f"v{st}")
                 for st in range(NKT)]

            def make_v(pool, st, tag=""):
                ps = pool.tile([128, HLOC, HD], F32, tag=tag or "psA", name="psv")
                for kt in range(NDT):
                    nc.tensor.matmul(
                        ps[:, :, :],
                        lhsT=xts[kt][:, 128 * st:128 * (st + 1)],
                        rhs=wv_sb[kt][:],
                        start=(kt == 0), stop=(kt == NDT - 1))
                nc.vector.tensor_add(V[st][:, :, :], ps[:, :, :],
                                     bvb_sb[:, :, :])

            def make_proj(pool, dst, wsb, col, hp, c, tag=""):
                # one 512-wide chunk of Q^T or K^T for head-pair hp
                ps = pool.tile([128, QC], F32, tag=tag or "psA", name="psp")
                for kt in range(NDT):
                    nc.tensor.matmul(
                        ps[:],
                        lhsT=wsb[kt][:, 128 * hp:128 * (hp + 1)],
                        rhs=xts[kt][:, QC * c:QC * (c + 1)],
                        start=(kt == 0), stop=(kt == NDT - 1))
                nc.vector.tensor_scalar_add(
                    dst[hp][:, QC * c:QC * (c + 1)], ps[:],
                    bqk_sb[:, col, hp:hp + 1])

            # ---- phase A: V (first NVUP tiles), K^T + Q^T chunk 0 ----
            with tc.tile_pool(name="psq", bufs=6, space="PSUM") as psq:
                for st in range(NVUP):
                    make_v(psq, st)
                for hp in range(NHP):
                    make_proj(psq, KT, wk_sb, 1, hp, 0)
                for hp in range(NHP):
                    make_proj(psq, QT, wq_sb, 0, hp, 0)

            # ---- phase B: attention with PE filler ----
            tri3 = trimask[:][:, None, :].broadcast_to([128, 2, 128])
            otn_store = {}
            filler = deque()

            with tc.tile_pool(name="pss", bufs=2, space="PSUM") as pss, \
                 tc.tile_pool(name="pst", bufs=2, space="PSUM") as pstp, \
                 tc.tile_pool(name="dnp", bufs=1, space="PSUM") as dnp, \
                 tc.tile_pool(name="flt", bufs=1, space="PSUM") as flt:

                def proj_unit(hp, c, dst, wsb, col):
                    # two parts: kt 0-3 (allocates the PSUM tile) and
                    # kt 4-7 + bias add (closes the accumulation group)
                    box = {}

                    def part1():
                        ps = flt.tile([128, QC], F32, tag="flt", name="psp")
                        box["ps"] = ps
                        for kt in range(4):
                            nc.tensor.matmul(
                                ps[:],
                                lhsT=wsb[kt][:, 128 * hp:128 * (hp + 1)],
                                rhs=xts[kt][:, QC * c:QC * (c + 1)],
                                start=(kt == 0), stop=False)

                    def part2():
                        ps = box["ps"]
                        for kt in range(4, NDT):
                            nc.tensor.matmul(
                                ps[:],
                                lhsT=wsb[kt][:, 128 * hp:128 * (hp + 1)],
                                rhs=xts[kt][:, QC * c:QC * (c + 1)],
                                start=False, stop=(kt == NDT - 1))
                        nc.vector.tensor_scalar_add(
                            dst[hp][:, QC * c:QC * (c + 1)], ps[:],
                            bqk_sb[:, col, hp:hp + 1])
                    return [(False, c, part1), (True, c, part2)]

                def v_unit(st):
                    box = {}

                    def part1():
                        ps = flt.tile([128, HLOC, HD], F32, tag="flt",
                                      name="psv")
                        box["ps"] = ps
                        for kt in range(4):
                            nc.tensor.matmul(
                                ps[:, :, :],
                                lhsT=xts[kt][:, 128 * st:128 * (st + 1)],
                                rhs=wv_sb[kt][:],
                                start=(kt == 0), stop=False)

                    def part2():
                        ps = box["ps"]
                        for kt in range(4, NDT):
                            nc.tensor.matmul(
                                ps[:, :, :],
                                lhsT=xts[kt][:, 128 * st:128 * (st + 1)],
                                rhs=wv_sb[kt][:],
                                start=False, stop=(kt == NDT - 1))
                        nc.vector.tensor_add(V[st][:, :, :], ps[:, :, :],
                                             bvb_sb[:, :, :])
                    return [(False, 3, part1), (True, 3, part2)]

                def oproj_unit(j, m, eo, pool=None):
                    s0 = QC * j + 128 * m

                    def go():
                        ps_o = (pool or flt).tile([128, 512], F32,
                                                  tag="flt" if pool is None
                                                  else "dnp", name="ps_o")
                        for hp in range(NHP):
                            nc.tensor.matmul(
                                ps_o[:],
                                lhsT=otn_store[(j, hp)][:,
                                                        128 * m:128 * (m + 1)],
                                rhs=wo_sb[hp][:, 512 * eo:512 * (eo + 1)],
                                start=(hp == 0), stop=(hp == NHP - 1))
                        osb = osbp.tile([128, 512], F32)
                        nc.vector.tensor_copy(osb[:], ps_o[:])
                        nc.sync.dma_start(
                            outp[s0:s0 + 128, 512 * eo:512 * (eo + 1)],
                            osb[:])
                    return [(False, None, go)]

                def pump(n):
                    for _ in range(n):
                        if not filler:
                            return
                        filler.popleft()[2]()

                def pump_flush_open():
                    # finish any unit whose PSUM accumulation group is
                    # still open so normalization may allocate from flt
                    while filler and filler[0][0]:
                        filler.popleft()[2]()

                def pump_unit():
                    # pop one FULL unit (never leaves a flt group open;
                    # safe to call between flt allocations)
                    if filler:
                        filler.popleft()[2]()
                    while filler and filler[0][0]:
                        filler.popleft()[2]()

                def flush_deadline(j):
                    # emit every part that must precede chains of step j
                    while filler and filler[0][1] is not None \
                            and filler[0][1] <= j:
                        filler.popleft()[2]()

                def chain(j, pair):
                    nkt = 4 * j + 4
                    hps = (2 * pair, 2 * pair + 1)
                    ps_t = {hp: pstp.tile([128, QC], F32, tag="pst",
                                          name=f"ps_t{hp}") for hp in hps}
                    denps = dnp.tile([128, QC], F32, tag="dnp", name="denps")
                    SKEW = 2
                    pts_hist = {}

                    def emit_av_hp(iv, hp):
                        wv_ = 128 * (iv - 4 * j) if iv >= 4 * j else 0
                        pt = pts_hist[iv][hp]
                        for h2 in range(2):
                            nc.tensor.matmul(
                                ps_t[hp][64 * h2:64 * (h2 + 1), wv_:QC],
                                lhsT=V[iv][:, 2 * hp + h2, :],
                                rhs=pt[:, h2, wv_:QC],
                                start=(iv == 0),
                                stop=(iv == nkt - 1))

                    def emit_den(iv):
                        wv_ = 128 * (iv - 4 * j) if iv >= 4 * j else 0
                        for qi, (hp, h2) in enumerate(
                                (hp, h2) for hp in hps for h2 in range(2)):
                            pt = pts_hist[iv][hp]
                            nc.tensor.matmul(
                                denps[32 * qi:32 * qi + 1, wv_:QC],
                                lhsT=ones1[:, 0:1],
                                rhs=pt[:, h2, wv_:QC],
                                start=(iv == 0),
                                stop=(iv == nkt - 1),
                                tile_position=(0, 32 * qi))
                        del pts_hist[iv]

                    for i in range(nkt):
                        w = 128 * (i - 4 * j) if i >= 4 * j else 0
                        pts = {}
                        for hp in hps:
                            ps_s = pss.tile([128, 2, QC], F32, tag="pss",
                                            name="ps_s")
                            for h2 in range(2):
                                nc.tensor.matmul(
                                    ps_s[:, h2, w:QC],
                                    lhsT=KT[hp][64 * h2:64 * (h2 + 1),
                                                128 * i:128 * (i + 1)],
                                    rhs=QT[hp][64 * h2:64 * (h2 + 1),
                                               QC * j + w:QC * (j + 1)],
                                    start=True, stop=True)
                            pt = ptp.tile([128, 2, QC], BF16, tag="pt",
                                          name="pt")
                            nc.scalar.activation(pt[:, :, w:QC],
                                                 ps_s[:, :, w:QC],
                                                 EXP, scale=SCALE)
                            if i >= 4 * j:
                                nc.vector.tensor_mul(
                                    pt[:, :, w:w + 128],
                                    pt[:, :, w:w + 128], tri3[:, :, :])
                            pts[hp] = pt
                        pts_hist[i] = pts
                        if i >= SKEW:
                            for hp in hps:
                                emit_av_hp(i - SKEW, hp)
                            emit_den(i - SKEW)
                        pump(1)
                    for iv in range(max(0, nkt - SKEW), nkt):
                        for hp in hps:
                            emit_av_hp(iv, hp)
                        emit_den(iv)
                    pump_flush_open()

                    # ---- normalization ----
                    # each denominator row -> its own [1,512] SBUF tile at
                    # partition 0 (32-aligned partition remap only), then
                    # reciprocal + bf16 cast, then one K=1 broadcast matmul
                    # per (hp, h2) into a full [128,512] PSUM bank.
                    quads = [(qi, hp, h2) for qi, (hp, h2) in enumerate(
                        (hp, h2) for hp in hps for h2 in range(2))]
                    den_rb = {}
                    for qi, hp, h2 in quads:
                        dsb = drp.tile([1, QC], F32, name="den_sb",
                                       tag="den")
                        nc.vector.tensor_copy(
                            dsb[:], denps[32 * qi:32 * qi + 1, :])
                        dr = drp.tile([1, QC], F32, name="den_r", tag="den")
                        nc.vector.reciprocal_approx_fast(dr[:], dsb[:])
                        drb = drp.tile([1, QC], BF16, name="den_rb",
                                       tag="den")
                        with nc.allow_low_precision(reason="denom"):
                            nc.vector.tensor_copy(drb[:], dr[:])
                        den_rb[hp, h2] = drb
                    pump_unit()
                    for hp in hps:
                        otn = otnp.tile([128, QC], BF16, tag="otn",
                                        name="otn")
                        rdps = flt.tile([128, QC], F32, tag="flt",
                                        name="rdps")
                        for h2 in range(2):
                            nc.tensor.matmul(
                                rdps[64 * h2:64 * (h2 + 1), :],
                                lhsT=onesr[:],
                                rhs=den_rb[hp, h2][:],
                                start=True, stop=True)
                        rdsb = rdsp.tile([128, QC], F32, name="rdsb")
                        nc.vector.tensor_copy(rdsb[:], rdps[:])
                        nc.vector.tensor_mul(otn[:], ps_t[hp][:], rdsb[:])
                        otn_store[(j, hp)] = otn
                        pump_unit()

                for j in range(NQC):
                    flush_deadline(j)
                    if j + 1 < NQC:
                        for hp in range(NHP):
                            filler.extend(
                                proj_unit(hp, j + 1, KT, wk_sb, 1))
                        for hp in range(NHP):
                            filler.extend(
                                proj_unit(hp, j + 1, QT, wq_sb, 0))
                    if j == 3:
                        for st in range(NVUP, NKT):
                            filler.extend(v_unit(st))
                    if j >= 1:
                        for m in range(4):
                            for eo in range(2):
                                filler.extend(oproj_unit(j - 1, m, eo))
                    chain(j, 0)
                    chain(j, 1)
                # drain remaining filler + final output projection
                # (alternate the two free PSUM pools to halve the tail
                # serialization on the single flt bank)
                while filler:
                    filler.popleft()[2]()
                for k, (m, eo) in enumerate(
                        (m, eo) for m in range(4) for eo in range(2)):
                    for _, _, part in oproj_unit(3, m, eo,
                                                 pool=dnp if k % 2 else None):
                        part()
    nc.compile()
    return nc


def _get_nc():
    if "nc" not in _CACHE:
        _CACHE["nc"] = _build_nc()
    return _CACHE["nc"]


def _prep_core_inputs(x, w_qkv, b_qkv, w_out, b, hg):
    r0 = ELOC * hg
    wq = w_qkv[r0:r0 + ELOC, :]
    wk = w_qkv[D + r0:D + r0 + ELOC, :]
    wv = w_qkv[2 * D + r0:2 * D + r0 + ELOC, :]
    bq = b_qkv[r0:r0 + ELOC]
    bk = b_qkv[D + r0:D + r0 + ELOC]
    bv = b_qkv[2 * D + r0:2 * D + r0 + ELOC]

    bf = ml_dtypes.bfloat16
    bqk_arr = np.empty((128, 2, NHP), np.float32)
    bqk_arr[:, 0, :] = bq.reshape(NHP, 128).T
    bqk_arr[:, 1, :] = bk.reshape(NHP, 128).T
    return {
        "xT": np.ascontiguousarray(x[b].T).astype(bf),
        "wqT": np.ascontiguousarray(wq.T).astype(bf),
        "wkT": np.ascontiguousarray(wk.T).astype(bf),
        "wvT": np.ascontiguousarray(wv.T).astype(bf),
        "woT": np.ascontiguousarray(w_out[:, r0:r0 + ELOC].T).astype(bf),
        "bqk": bqk_arr,
        "bvb": np.tile(bv.astype(np.float32)[None, :],
                       (128, 1)).reshape(128, HLOC, HD),
    }


def kernel(x, w_qkv, b_qkv, w_out, b_out, _trace=False, _trace_kwargs=None):
    x = np.asarray(x, np.float32)
    w_qkv = np.asarray(w_qkv, np.float32)
    b_qkv = np.asarray(b_qkv, np.float32)
    w_out = np.asarray(w_out, np.float32)
    b_out = np.asarray(b_out, np.float32)

    nc = _get_nc()
    in_maps = []
    for core in range(NCORES):
        b, hg = core // 2, core % 2
        in_maps.append(_prep_core_inputs(x, w_qkv, b_qkv, w_out, b, hg))

    kw = {}
    if _trace:
        kw.update(trace=True, **(_trace_kwargs or {}))
    import time
    res = None
    for attempt in range(4):
        try:
            res = bass_utils.run_bass_kernel_spmd(
                nc, in_maps, core_ids=list(range(NCORES)), **kw)
            break
        except Exception:
            if attempt == 3:
                raise
            # Transient axon/NRT device flake: reset the PJRT backend so the
            # retry starts from a clean client, like a fresh process would.
            try:
                import jax
                jax.clear_caches()
                import jax._src.xla_bridge as _xb
                _xb._clear_backends()
            except Exception:
                pass
            time.sleep(5.0 * (attempt + 1))

    out = np.empty((B, S, D), np.float32)
    for b in range(B):
        out[b] = res.results[2 * b]["outp"] + res.results[2 * b + 1]["outp"] \
            + b_out[None, :]
    if _trace:
        return out, res
    return out


# revision 14
# speedup vs baseline: 1.2808x; 1.0198x over previous
"""Causal multi-head attention (dense transformer block) on 8 Trainium2 cores.

Problem: x[4, 2048, 1024], 16 heads, head_dim 64, causal softmax attention
with QKV + output projections (torch Linear layout weights).

Sharding: 8 cores = 4 batches x 2 head-groups (8 heads each).  Each core
computes QKV projection for its 8 heads, attention, and its partial output
projection (row-parallel over w_out).  Host sums the two partials per batch
and adds b_out.

Device layouts are "transposed" so no on-device transposes are needed:
  - x is fed as xT [d, s]; Q^T/K^T are produced as [head_dim, s]
  - scores are computed as S^T [k, q]; the two heads of a pair run as
    row-group-tiled concurrent matmuls (K=64 contraction at array rows
    0-63 / 64-127).
  - AV is col-group packed: per head-pair one PSUM bank holds O^T for
    head A in partitions 0..63 and head B in partitions 64..127, written
    by two concurrent col-tiled matmuls (tile_position auto-derived).
  - softmax denominators come from four col-packed M=1 ones-matmuls per
    i-step accumulating into rows 0/32/64/96 of a dedicated PSUM bank.
  - normalization: denominator rows are copied (partition-remapped) to
    SBUF, reciprocal'd at [2,512] cost, broadcast into a full 128-row
    PSUM bank by one K=2 selector matmul per hp, copied to SBUF, and
    applied with one [128,512] DVE multiply per hp.
  - PE filler: the deferred Q-chunk projections (j>=1), deferred V tiles
    (st>=12), and the output projections are emitted *between* attention
    i-steps so the tensor engine never idles while the scalar engine
    (exp) catches up -- this also keeps the PE HAM clock un-throttled.
Matmul inputs are bf16 (PSUM accumulation is fp32); everything else fp32.
"""

import sys

sys.path.insert(0, "/opt/trn_rl_repo")

from collections import deque

import numpy as np
import ml_dtypes

import concourse.bass as bass
import concourse.mybir as mybir
import concourse.tile as tile
from concourse import bacc
from concourse import bass_utils
from concourse.masks import make_upper_triangular

F32 = mybir.dt.float32
BF16 = mybir.dt.bfloat16
EXP = mybir.ActivationFunctionType.Exp

B, S, D = 4, 2048, 1024
HTOT, HD = 16, 64
NCORES = 8
HLOC = HTOT // 2          # heads per core
ELOC = HLOC * HD          # 512 local embedding width
NHP = HLOC // 2           # 4 head pairs
QC = 512                  # q-chunk width
NQC = S // QC             # 4
NKT = S // 128            # 16 k tiles over sequence
NDT = D // 128            # 8 k tiles over model dim
SCALE = 1.0 / float(np.sqrt(HD))
NVUP = 12                 # V s-tiles computed upfront; the rest are filler

_CACHE = {}


def _build_nc():
    nc = bacc.Bacc("TRN2", target_bir_lowering=False, debug=False)

    xT = nc.dram_tensor("xT", [D, S], BF16, kind="ExternalInput")
    wqT = nc.dram_tensor("wqT", [D, ELOC], BF16, kind="ExternalInput")
    wkT = nc.dram_tensor("wkT", [D, ELOC], BF16, kind="ExternalInput")
    wvT = nc.dram_tensor("wvT", [D, ELOC], BF16, kind="ExternalInput")
    woT = nc.dram_tensor("woT", [ELOC, D], BF16, kind="ExternalInput")
    bqk = nc.dram_tensor("bqk", [128, 2, NHP], F32, kind="ExternalInput")
    bvb = nc.dram_tensor("bvb", [128, HLOC, HD], F32, kind="ExternalInput")
    outp = nc.dram_tensor("outp", [S, D], F32, kind="ExternalOutput")

    with tile.TileContext(nc) as tc:
        with tc.tile_pool(name="const", bufs=1) as constp, \
             tc.tile_pool(name="wpool", bufs=1) as wp, \
             tc.tile_pool(name="qkv", bufs=1) as qkvp, \
             tc.tile_pool(name="xt", bufs=1) as xtp, \
             tc.tile_pool(name="pt", bufs=8) as ptp, \
             tc.tile_pool(name="otn", bufs=8) as otnp, \
             tc.tile_pool(name="dr", bufs=8) as drp, \
             tc.tile_pool(name="rds", bufs=2) as rdsp, \
             tc.tile_pool(name="osb", bufs=4) as osbp:

            # ---- constants ----
            trimask = constp.tile([128, 128], BF16, name="trimask")
            make_upper_triangular(nc, trimask[:], val=1.0, diag=True)
            ones1 = constp.tile([128, 1], BF16, name="ones1")
            nc.gpsimd.memset(ones1[:], 1.0)
            # ones row for the K=1 denominator-broadcast matmuls
            onesr = constp.tile([1, 64], BF16, name="onesr")
            nc.gpsimd.memset(onesr[:], 1.0)

            # ---- weights + xT (V inputs first so compute starts early;
            # x is streamed in s-chunks interleaved with the wv tiles) ----
            wv_sb = [wp.tile([128, ELOC], BF16, name=f"wv{kt}")
                     for kt in range(NDT)]
            xts = [xtp.tile([128, S], BF16, name=f"xt{kt}")
                   for kt in range(NDT)]
            for kt in range(NDT):
                nc.sync.dma_start(wv_sb[kt][:],
                                  wvT[128 * kt:128 * (kt + 1), :])
                nc.sync.dma_start(
                    xts[kt][:, 0:128], xT[128 * kt:128 * (kt + 1), 0:128])
            for kt in range(NDT):
                nc.sync.dma_start(
                    xts[kt][:, 128:512], xT[128 * kt:128 * (kt + 1), 128:512])
            bqk_sb = constp.tile([128, 2, NHP], F32, name="bqk_sb")
            nc.sync.dma_start(bqk_sb[:], bqk[:])
            bvb_sb = constp.tile([128, HLOC, HD], F32, name="bvb_sb")
            nc.sync.dma_start(bvb_sb[:], bvb[:])
            for c in range(1, 4):
                for kt in range(NDT):
                    nc.sync.dma_start(
                        xts[kt][:, 512 * c:512 * (c + 1)],
                        xT[128 * kt:128 * (kt + 1), 512 * c:512 * (c + 1)])
            wq_sb, wk_sb = [], []
            for kt in range(NDT):
                for lst, srct, nm in ((wq_sb, wqT, "wq"), (wk_sb, wkT, "wk")):
                    t = wp.tile([128, ELOC], BF16, name=f"{nm}{kt}")
                    nc.sync.dma_start(t[:], srct[128 * kt:128 * (kt + 1), :])
                    lst.append(t)
            wo_sb = []
            for hp in range(NHP):
                t = wp.tile([128, D], BF16, name=f"wo{hp}")
                nc.sync.dma_start(t[:], woT[128 * hp:128 * (hp + 1), :])
                wo_sb.append(t)

            # ---- QKV projection tiles ----
            QT, KT = [], []
            for hp in range(NHP):
                QT.append(qkvp.tile([128, S], BF16, name=f"qt{hp}"))
                KT.append(qkvp.tile([128, S], BF16, name=f"kt{hp}"))
            V = [qkvp.tile([128, HLOC, HD], BF16, name=f"v{st}")
                 for st in range(NKT)]

            def make_v(pool, st, tag=""):
                ps = pool.tile([128, HLOC, HD], F32, tag=tag or "psA", name="psv")
                for kt in range(NDT):
                    nc.tensor.matmul(
                        ps[:, :, :],
                        lhsT=xts[kt][:, 128 * st:128 * (st + 1)],
                        rhs=wv_sb[kt][:],
                        start=(kt == 0), stop=(kt == NDT - 1))
                nc.vector.tensor_add(V[st][:, :, :], ps[:, :, :],
                                     bvb_sb[:, :, :])

            def make_proj(pool, dst, wsb, col, hp, c, tag=""):
                # one 512-wide chunk of Q^T or K^T for head-pair hp
                ps = pool.tile([128, QC], F32, tag=tag or "psA", name="psp")
                for kt in range(NDT):
                    nc.tensor.matmul(
                        ps[:],
                        lhsT=wsb[kt][:, 128 * hp:128 * (hp + 1)],
                        rhs=xts[kt][:, QC * c:QC * (c + 1)],
                        start=(kt == 0), stop=(kt == NDT - 1))
                nc.vector.tensor_scalar_add(
                    dst[hp][:, QC * c:QC * (c + 1)], ps[:],
                    bqk_sb[:, col, hp:hp + 1])

            # ---- phase A: V (first NVUP tiles), K^T + Q^T chunk 0 ----
            with tc.tile_pool(name="psq", bufs=6, space="PSUM") as psq:
                for st in range(NVUP):
                    make_v(psq, st)
                for hp in range(NHP):
                    make_proj(psq, KT, wk_sb, 1, hp, 0)
                for hp in range(NHP):
                    make_proj(psq, QT, wq_sb, 0, hp, 0)

            # ---- phase B: attention with PE filler ----
            tri3 = trimask[:][:, None, :].broadcast_to([128, 2, 128])
            otn_store = {}
            filler = deque()
            reserve = deque()
            pts_store = {}

            with tc.tile_pool(name="pss", bufs=2, space="PSUM") as pss, \
                 tc.tile_pool(name="pst", bufs=2, space="PSUM") as pstp, \
                 tc.tile_pool(name="dnp", bufs=1, space="PSUM") as dnp, \
                 tc.tile_pool(name="flt", bufs=1, space="PSUM") as flt:

                def proj_unit(hp, c, dst, wsb, col):
                    # two parts: kt 0-3 (allocates the PSUM tile) and
                    # kt 4-7 + bias add (closes the accumulation group)
                    box = {}

                    def part1():
                        ps = flt.tile([128, QC], F32, tag="flt", name="psp")
                        box["ps"] = ps
                        for kt in range(4):
                            nc.tensor.matmul(
                                ps[:],
                                lhsT=wsb[kt][:, 128 * hp:128 * (hp + 1)],
                                rhs=xts[kt][:, QC * c:QC * (c + 1)],
                                start=(kt == 0), stop=False)

                    def part2():
                        ps = box["ps"]
                        for kt in range(4, NDT):
                            nc.tensor.matmul(
                                ps[:],
                                lhsT=wsb[kt][:, 128 * hp:128 * (hp + 1)],
                                rhs=xts[kt][:, QC * c:QC * (c + 1)],
                                start=False, stop=(kt == NDT - 1))
                        nc.vector.tensor_scalar_add(
                            dst[hp][:, QC * c:QC * (c + 1)], ps[:],
                            bqk_sb[:, col, hp:hp + 1])
                    return [(False, c, part1), (True, c, part2)]

                def v_unit(st):
                    box = {}

                    def part1():
                        ps = flt.tile([128, HLOC, HD], F32, tag="flt",
                                      name="psv")
                        box["ps"] = ps
                        for kt in range(4):
                            nc.tensor.matmul(
                                ps[:, :, :],
                                lhsT=xts[kt][:, 128 * st:128 * (st + 1)],
                                rhs=wv_sb[kt][:],
                                start=(kt == 0), stop=False)

                    def part2():
                        ps = box["ps"]
                        for kt in range(4, NDT):
                            nc.tensor.matmul(
                                ps[:, :, :],
                                lhsT=xts[kt][:, 128 * st:128 * (st + 1)],
                                rhs=wv_sb[kt][:],
                                start=False, stop=(kt == NDT - 1))
                        nc.vector.tensor_add(V[st][:, :, :], ps[:, :, :],
                                             bvb_sb[:, :, :])
                    return [(False, 3, part1), (True, 3, part2)]

                def oproj_unit(j, m, eo, pool=None):
                    s0 = QC * j + 128 * m

                    def go():
                        ps_o = (pool or flt).tile([128, 512], F32,
                                                  tag="flt" if pool is None
                                                  else "dnp", name="ps_o")
                        for hp in range(NHP):
                            nc.tensor.matmul(
                                ps_o[:],
                                lhsT=otn_store[(j, hp)][:,
                                                        128 * m:128 * (m + 1)],
                                rhs=wo_sb[hp][:, 512 * eo:512 * (eo + 1)],
                                start=(hp == 0), stop=(hp == NHP - 1))
                        osb = osbp.tile([128, 512], F32)
                        nc.vector.tensor_copy(osb[:], ps_o[:])
                        nc.sync.dma_start(
                            outp[s0:s0 + 128, 512 * eo:512 * (eo + 1)],
                            osb[:])
                    return [(False, None, go)]

                def pump(n):
                    for _ in range(n):
                        if filler:
                            filler.popleft()[2]()
                        elif reserve:
                            reserve.popleft()[2]()
                        else:
                            return

                def pump_flush_open():
                    # finish any unit whose PSUM accumulation group is
                    # still open so normalization may allocate from flt
                    while filler and filler[0][0]:
                        filler.popleft()[2]()

                def pump_unit():
                    # pop one FULL unit (never leaves a flt group open;
                    # safe to call between flt allocations)
                    if filler:
                        filler.popleft()[2]()
                        while filler and filler[0][0]:
                            filler.popleft()[2]()
                    elif reserve:
                        reserve.popleft()[2]()
                        while reserve and reserve[0][0]:
                            reserve.popleft()[2]()

                def flush_deadline(j):
                    # emit every part that must precede chains of step j
                    while filler and filler[0][1] is not None \
                            and filler[0][1] <= j:
                        filler.popleft()[2]()

                def emit_step(j, pair, i, pts_hist):
                    # scores + exp + mask for one (chain, i) -- no AV
                    hps = (2 * pair, 2 * pair + 1)
                    w = 128 * (i - 4 * j) if i >= 4 * j else 0
                    pts = {}
                    for hp in hps:
                        ps_s = pss.tile([128, 2, QC], F32, tag="pss",
                                        name="ps_s")
                        for h2 in range(2):
                            nc.tensor.matmul(
                                ps_s[:, h2, w:QC],
                                lhsT=KT[hp][64 * h2:64 * (h2 + 1),
                                            128 * i:128 * (i + 1)],
                                rhs=QT[hp][64 * h2:64 * (h2 + 1),
                                           QC * j + w:QC * (j + 1)],
                                start=True, stop=True)
                        pt = ptp.tile([128, 2, QC], BF16, tag="pt",
                                      name="pt")
                        nc.scalar.activation(pt[:, :, w:QC],
                                             ps_s[:, :, w:QC],
                                             EXP, scale=SCALE)
                        if i >= 4 * j:
                            nc.vector.tensor_mul(
                                pt[:, :, w:w + 128],
                                pt[:, :, w:w + 128], tri3[:, :, :])
                        pts[hp] = pt
                    pts_hist[i] = pts

                def chain(j, pair, pre=0, prologue=None):
                    nkt = 4 * j + 4
                    hps = (2 * pair, 2 * pair + 1)
                    ps_t = {hp: pstp.tile([128, QC], F32, tag="pst",
                                          name=f"ps_t{hp}") for hp in hps}
                    denps = dnp.tile([128, QC], F32, tag="dnp", name="denps")
                    SKEW = 2
                    pts_hist = pts_store.pop((j, pair), {})

                    def emit_av_hp(iv, hp):
                        wv_ = 128 * (iv - 4 * j) if iv >= 4 * j else 0
                        pt = pts_hist[iv][hp]
                        for h2 in range(2):
                            nc.tensor.matmul(
                                ps_t[hp][64 * h2:64 * (h2 + 1), wv_:QC],
                                lhsT=V[iv][:, 2 * hp + h2, :],
                                rhs=pt[:, h2, wv_:QC],
                                start=(iv == 0),
                                stop=(iv == nkt - 1))

                    def emit_den(iv):
                        wv_ = 128 * (iv - 4 * j) if iv >= 4 * j else 0
                        for qi, (hp, h2) in enumerate(
                                (hp, h2) for hp in hps for h2 in range(2)):
                            pt = pts_hist[iv][hp]
                            nc.tensor.matmul(
                                denps[32 * qi:32 * qi + 1, wv_:QC],
                                lhsT=ones1[:, 0:1],
                                rhs=pt[:, h2, wv_:QC],
                                start=(iv == 0),
                                stop=(iv == nkt - 1),
                                tile_position=(0, 32 * qi))
                        del pts_hist[iv]

                    for i in range(pre, nkt):
                        emit_step(j, pair, i, pts_hist)
                        if i >= SKEW:
                            for hp in hps:
                                emit_av_hp(i - SKEW, hp)
                            emit_den(i - SKEW)
                        pump(1)
                    for iv in range(max(0, nkt - SKEW), nkt):
                        for hp in hps:
                            emit_av_hp(iv, hp)
                        emit_den(iv)
                    pump_flush_open()
                    if prologue is not None:
                        prologue()

                    # ---- normalization ----
                    # each denominator row -> its own [1,512] SBUF tile at
                    # partition 0 (32-aligned partition remap only), then
                    # reciprocal + bf16 cast, then one K=1 broadcast matmul
                    # per (hp, h2) into a full [128,512] PSUM bank.
                    quads = [(qi, hp, h2) for qi, (hp, h2) in enumerate(
                        (hp, h2) for hp in hps for h2 in range(2))]
                    den_rb = {}
                    for qi, hp, h2 in quads:
                        dsb = drp.tile([1, QC], F32, name="den_sb",
                                       tag="den")
                        nc.vector.tensor_copy(
                            dsb[:], denps[32 * qi:32 * qi + 1, :])
                        dr = drp.tile([1, QC], F32, name="den_r", tag="den")
                        nc.vector.reciprocal_approx_fast(dr[:], dsb[:])
                        drb = drp.tile([1, QC], BF16, name="den_rb",
                                       tag="den")
                        with nc.allow_low_precision(reason="denom"):
                            nc.vector.tensor_copy(drb[:], dr[:])
                        den_rb[hp, h2] = drb
                    pump_unit()
                    for hp in hps:
                        otn = otnp.tile([128, QC], BF16, tag="otn",
                                        name="otn")
                        rdps = flt.tile([128, QC], F32, tag="flt",
                                        name="rdps")
                        for h2 in range(2):
                            nc.tensor.matmul(
                                rdps[64 * h2:64 * (h2 + 1), :],
                                lhsT=onesr[:],
                                rhs=den_rb[hp, h2][:],
                                start=True, stop=True)
                        rdsb = rdsp.tile([128, QC], F32, name="rdsb")
                        nc.vector.tensor_copy(rdsb[:], rdps[:])
                        nc.vector.tensor_mul(otn[:], ps_t[hp][:], rdsb[:])
                        otn_store[(j, hp)] = otn
                        pump_unit()

                chain_list = [(j, p) for j in range(NQC) for p in (0, 1)]
                PRE = 2

                def make_prologue(idx):
                    if idx + 1 >= len(chain_list):
                        return None
                    nj, np_ = chain_list[idx + 1]

                    def prologue():
                        flush_deadline(nj)
                        ph = pts_store.setdefault((nj, np_), {})
                        for i in range(PRE):
                            emit_step(nj, np_, i, ph)
                    return prologue

                for idx, (j, pair) in enumerate(chain_list):
                    if pair == 0:
                        flush_deadline(j)
                        if j + 1 < NQC:
                            for hp in range(NHP):
                                filler.extend(
                                    proj_unit(hp, j + 1, KT, wk_sb, 1))
                            for hp in range(NHP):
                                filler.extend(
                                    proj_unit(hp, j + 1, QT, wq_sb, 0))
                        if j == 3:
                            for st in range(NVUP, NKT):
                                filler.extend(v_unit(st))
                        if j >= 1:
                            for k, (m, eo) in enumerate(
                                    (m, eo) for m in range(4)
                                    for eo in range(2)):
                                dstq = reserve if (j == 3 and k >= 4) \
                                    else filler
                                dstq.extend(oproj_unit(j - 1, m, eo))
                    chain(j, pair, pre=(PRE if idx > 0 else 0),
                          prologue=make_prologue(idx))
                # drain remaining filler + final output projection
                # (alternate the two free PSUM pools to halve the tail
                # serialization on the single flt bank)
                while filler:
                    filler.popleft()[2]()
                while reserve:
                    reserve.popleft()[2]()
                for k, (m, eo) in enumerate(
                        (m, eo) for m in range(4) for eo in range(2)):
                    for _, _, part in oproj_unit(3, m, eo,
                                                 pool=dnp if k % 2 else None):
                        part()
    nc.compile()
    return nc


def _get_nc():
    if "nc" not in _CACHE:
        _CACHE["nc"] = _build_nc()
    return _CACHE["nc"]


def _prep_core_inputs(x, w_qkv, b_qkv, w_out, b, hg):
    r0 = ELOC * hg
    wq = w_qkv[r0:r0 + ELOC, :]
    wk = w_qkv[D + r0:D + r0 + ELOC, :]
    wv = w_qkv[2 * D + r0:2 * D + r0 + ELOC, :]
    bq = b_qkv[r0:r0 + ELOC]
    bk = b_qkv[D + r0:D + r0 + ELOC]
    bv = b_qkv[2 * D + r0:2 * D + r0 + ELOC]

    bf = ml_dtypes.bfloat16
    bqk_arr = np.empty((128, 2, NHP), np.float32)
    bqk_arr[:, 0, :] = bq.reshape(NHP, 128).T
    bqk_arr[:, 1, :] = bk.reshape(NHP, 128).T
    return {
        "xT": np.ascontiguousarray(x[b].T).astype(bf),
        "wqT": np.ascontiguousarray(wq.T).astype(bf),
        "wkT": np.ascontiguousarray(wk.T).astype(bf),
        "wvT": np.ascontiguousarray(wv.T).astype(bf),
        "woT": np.ascontiguousarray(w_out[:, r0:r0 + ELOC].T).astype(bf),
        "bqk": bqk_arr,
        "bvb": np.tile(bv.astype(np.float32)[None, :],
                       (128, 1)).reshape(128, HLOC, HD),
    }


def kernel(x, w_qkv, b_qkv, w_out, b_out, _trace=False, _trace_kwargs=None):
    x = np.asarray(x, np.float32)
    w_qkv = np.asarray(w_qkv, np.float32)
    b_qkv = np.asarray(b_qkv, np.float32)
    w_out = np.asarray(w_out, np.float32)
    b_out = np.asarray(b_out, np.float32)

    nc = _get_nc()
    in_maps = []
    for core in range(NCORES):
        b, hg = core // 2, core % 2
        in_maps.append(_prep_core_inputs(x, w_qkv, b_qkv, w_out, b, hg))

    kw = {}
    if _trace:
        kw.update(trace=True, **(_trace_kwargs or {}))
    import time
    res = None
    for attempt in range(4):
        try:
            res = bass_utils.run_bass_kernel_spmd(
                nc, in_maps, core_ids=list(range(NCORES)), **kw)
            break
        except Exception:
            if attempt == 3:
                raise
            # Transient axon/NRT device flake: reset the PJRT backend so the
            # retry starts from a clean client, like a fresh process would.
            try:
                import jax
                jax.clear_caches()
                import jax._src.xla_bridge as _xb
                _xb._clear_backends()
            except Exception:
                pass
            time.sleep(5.0 * (attempt + 1))

    out = np.empty((B, S, D), np.float32)
    for b in range(B):
        out[b] = res.results[2 * b]["outp"] + res.results[2 * b + 1]["outp"] \
            + b_out[None, :]
    if _trace:
        return out, res
    return out
